# revision 1
# baseline (speedup 1.0000x reference)
"""Trainium2 Bass kernel for MFVIConstituency mean-field iterations.

Per batch b (one NeuronCore each, 8 total):
    q = s_con;  repeat 3x:  q[i,j] = s_con[i,j] + sum_k sig(q)[j,k] * sb[i,j,k]
    out = sigmoid(q)
where sb = s_bin * mask2o, mask2o[i,j,k] = mask[i,j] & (i!=k) & (j!=k).

Host (numpy) does: masking, fp16 cast, SBUF-cache layout packing, iteration-1
sigmoid, final transpose. Device does, per iteration: fp16 tensor_tensor mul
(DVE 2x mode) -> segmented reduction split between a DVE in-place pairwise
tree (fp16 adds at 2x) and ACT activation-accumulate, then sigmoid + xbar
transposes to rebuild the sig operand layout.

On-chip layout: q is assembled transposed (QT[j,i]); j lives on partitions in
two chunks: chunk1 = j 0:128, chunk2 "packed" = j 128:192 duplicated across
both partition halves with the i-range split (p<64: i 0:96, p>=64: i 96:192)
so every DVE instruction uses all 128 partitions.
"""

import numpy as np

S = 192
B = 8
P = 128
G = 48          # i-values per slab -> slab free size G*S = 9216
NSLAB1 = 4      # chunk1: 4 slabs of 48 i-values (j in 0:128)
NSLAB2 = 2      # chunk2 packed: 96 i-per-half * 2 halves / 48
DVE_SEGS = 34   # per slab: segments reduced by the DVE tree; rest go to ACT
SLAB_ORDER = [4, 5, 0, 1, 2, 3]   # chunk2 first so its boundary work overlaps

_CACHE = {}


def _build_program():
    import concourse.tile as tile
    from concourse import mybir, bacc
    from contextlib import ExitStack

    f32, f16 = mybir.dt.float32, mybir.dt.float16
    SLAB = G * S
    Sig = None

    nc = bacc.Bacc("TRN2", target_bir_lowering=False, debug=False, num_devices=B)
    Sig = __import__("concourse.mybir", fromlist=["x"]).ActivationFunctionType.Sigmoid
    Cpy = __import__("concourse.mybir", fromlist=["x"]).ActivationFunctionType.Copy
    c1_d = nc.dram_tensor("c1", [P, NSLAB1 * SLAB], f16, kind="ExternalInput")
    c2_d = nc.dram_tensor("c2", [P, NSLAB2 * SLAB], f16, kind="ExternalInput")
    siga_d = nc.dram_tensor("siga", [P, S], f16, kind="ExternalInput")
    sigb_d = nc.dram_tensor("sigb", [P, S], f16, kind="ExternalInput")
    sconT1_d = nc.dram_tensor("sconT1", [P, S], f32, kind="ExternalInput")
    sconT2p_d = nc.dram_tensor("sconT2p", [P, 96], f32, kind="ExternalInput")
    qt_d = nc.dram_tensor("qt_out", [S, S], f32, kind="ExternalOutput")

    with tile.TileContext(nc) as tc, ExitStack() as ctx:
        cache_p = ctx.enter_context(tc.tile_pool(name="cache", bufs=1))
        small_p = ctx.enter_context(tc.tile_pool(name="small", bufs=1))
        sig_p = ctx.enter_context(tc.tile_pool(name="sig", bufs=2))
        qt_p = ctx.enter_context(tc.tile_pool(name="qt", bufs=2))
        p_p = ctx.enter_context(tc.tile_pool(name="prod", bufs=4))
        junk_p = ctx.enter_context(tc.tile_pool(name="junk", bufs=4))
        sq_p = ctx.enter_context(tc.tile_pool(name="sq", bufs=2))
        out_p = ctx.enter_context(tc.tile_pool(name="out", bufs=1))

        sconT1_t = small_p.tile([P, S], f32, tag="sc1")
        nc.scalar.dma_start(sconT1_t[:], sconT1_d.ap())
        sconT2p_t = small_p.tile([P, 96], f32, tag="sc2")
        nc.scalar.dma_start(sconT2p_t[:], sconT2p_d.ap())
        siga_t = sig_p.tile([P, S], f16, tag="siga")
        nc.scalar.dma_start(siga_t[:], siga_d.ap())
        sigb_t = sig_p.tile([P, S], f16, tag="sigb")
        nc.scalar.dma_start(sigb_t[:], sigb_d.ap())

        cts = {}
        for idx, s in enumerate(SLAB_ORDER):
            ct = cache_p.tile([P, SLAB], f16, tag=f"c{s}")
            if s < NSLAB1:
                src = c1_d.ap()[:, s * SLAB:(s + 1) * SLAB]
            else:
                src = c2_d.ap()[:, (s - NSLAB1) * SLAB:(s - NSLAB1 + 1) * SLAB]
            eng = nc.sync
            if idx < 2:
                # split first-wave loads so compute ramps sooner
                h = SLAB // 2
                eng.dma_start(ct[:, 0:h], src[:, 0:h])
                eng.dma_start(ct[:, h:SLAB], src[:, h:SLAB])
            else:
                eng.dma_start(ct[:], src)
            cts[s] = ct

        def do_slab(s, siga_t, sigb_t, qt1, qt2, split=None):
            is1 = s < NSLAB1
            sig_t = siga_t if is1 else sigb_t
            qt_t = qt1 if is1 else qt2
            base = (s if is1 else s - NSLAB1) * G
            pt = p_p.tile([P, SLAB], f16)
            p3 = pt[:].rearrange("p (g k) -> p g k", k=S)
            in0 = cts[s][:].rearrange("p (g k) -> p g k", k=S)
            in1 = sig_t[:].unsqueeze(1).broadcast_to([P, G, S])
            if split == "g":       # ramp: match the halved first-wave DMAs
                h = G // 2
                nc.vector.tensor_tensor(p3[:, 0:h, :], in0[:, 0:h, :],
                                        in1[:, 0:h, :], mybir.AluOpType.mult)
                nc.vector.tensor_tensor(p3[:, h:G, :], in0[:, h:G, :],
                                        in1[:, h:G, :], mybir.AluOpType.mult)
            elif split == "k":     # boundary: high k-columns are ready first
                nc.vector.tensor_tensor(p3[:, :, 128:S], in0[:, :, 128:S],
                                        in1[:, :, 128:S], mybir.AluOpType.mult)
                nc.vector.tensor_tensor(p3[:, :, 0:128], in0[:, :, 0:128],
                                        in1[:, :, 0:128], mybir.AluOpType.mult)
            else:
                nc.vector.tensor_tensor(p3, in0, in1, mybir.AluOpType.mult)
            d = DVE_SEGS
            if d > 0:
                w = S
                while w > 3:   # in-place fp16 pairwise tree: 192->96->...->3
                    h = w // 2
                    nc.vector.tensor_tensor(
                        p3[:, 0:d, 0:h], p3[:, 0:d, 0:h], p3[:, 0:d, h:w],
                        mybir.AluOpType.add)
                    w = h
                nc.vector.tensor_reduce(
                    qt_t[:, base:base + d], p3[:, 0:d, 0:3],
                    axis=mybir.AxisListType.X, op=mybir.AluOpType.add)
            for g in range(d, G):
                jt = junk_p.tile([P, S], f16)
                nc.scalar.activation(
                    jt[:], pt[:, g * S:(g + 1) * S], Cpy,
                    accum_out=qt_t[:, base + g:base + g + 1])

        for it in range(3):
            qt1 = qt_p.tile([P, S], f32, tag="qt1")
            qt2 = qt_p.tile([P, 96], f32, tag="qt2")
            last = it == 2
            if not last:
                nsa = sig_p.tile([P, S], f16, tag="siga")
                nsb = sig_p.tile([P, S], f16, tag="sigb")
                sq1 = sq_p.tile([P, 256], f16, tag="sq1")
                sq2 = sq_p.tile([P, 128], f16, tag="sq2")
                tmp1 = sq_p.tile([P, 128], f16, tag="tmp1")
                tmp2 = sq_p.tile([P, 128], f16, tag="tmp2")

            for si, s in enumerate(SLAB_ORDER[0:2]):   # chunk2 slabs first
                sp = "g" if it == 0 else ("k" if si == 0 else None)
                do_slab(s, siga_t, sigb_t, qt1, qt2, split=sp)
            nc.vector.tensor_tensor(qt2[:], qt2[:], sconT2p_t[:], mybir.AluOpType.add)
            if not last:
                # chunk2 boundary work overlaps chunk1 compute below
                nc.scalar.activation(sq2[:, 0:96], qt2[:], Sig)
                nc.scalar.activation(sq2[:, 96:128], qt2[:, 0:32], Sig)  # filler
                nc.sync.dma_start_transpose(tmp2[:], sq2[:])
                nc.scalar.dma_start(nsa[0:96, 128:192], tmp2[0:96, 0:64])
                nc.scalar.dma_start(nsa[96:128, 128:192], tmp2[0:32, 64:128])
                nc.scalar.dma_start(nsb[0:64, 128:192], tmp2[32:96, 64:128])
                nc.scalar.dma_start(nsb[64:128, 128:192], tmp2[32:96, 64:128])
            else:
                o2 = out_p.tile([P, 96], f32, tag="o2")
                nc.scalar.activation(o2[:], qt2[:], Sig)
                nc.sync.dma_start(qt_d.ap()[128:192, 0:96], o2[0:64, :])
                nc.sync.dma_start(qt_d.ap()[128:192, 96:192], o2[64:128, :])

            for s in SLAB_ORDER[2:]:            # chunk1 slabs
                do_slab(s, siga_t, sigb_t, qt1, qt2)
            nc.vector.tensor_tensor(qt1[:], qt1[:], sconT1_t[:], mybir.AluOpType.add)
            if not last:
                nc.scalar.activation(sq1[:, 0:S], qt1[:], Sig)
                nc.scalar.activation(sq1[:, S:256], qt1[:, 0:64], Sig)  # filler
                nc.sync.dma_start_transpose(nsa[0:128, 0:128], sq1[:, 0:128])
                nc.sync.dma_start_transpose(tmp1[:], sq1[:, 128:256])
                nc.scalar.dma_start(nsb[0:64, 0:128], tmp1[0:64, :])
                nc.scalar.dma_start(nsb[64:128, 0:128], tmp1[0:64, :])
                siga_t, sigb_t = nsa, nsb
            else:
                o1 = out_p.tile([P, S], f32, tag="o1")
                nc.scalar.activation(o1[:], qt1[:], Sig)
                nc.sync.dma_start(qt_d.ap()[0:128, :], o1[:])
    nc.compile()
    return nc


def _get_program():
    if "nc" not in _CACHE:
        _CACHE["nc"] = _build_program()
    return _CACHE["nc"]


def _prep_core_inputs(s_con_b, sbm16_b):
    """Per-batch input dict. sbm16_b: masked s_bin, fp16, [i, j, k]."""
    A = sbm16_b
    c1 = np.ascontiguousarray(A[:, 0:128, :].transpose(1, 0, 2)).reshape(P, S * S)
    c2a = A[0:96, 128:192, :].transpose(1, 0, 2)     # [64, 96, 192]
    c2b = A[96:192, 128:192, :].transpose(1, 0, 2)   # [64, 96, 192]
    c2 = np.ascontiguousarray(np.concatenate([c2a, c2b], 0)).reshape(P, 96 * S)
    sig1 = (1.0 / (1.0 + np.exp(-s_con_b))).astype(np.float16)   # [a, k] natural
    siga = np.ascontiguousarray(sig1[0:128])
    sigb = np.ascontiguousarray(np.concatenate([sig1[128:192]] * 2, 0))
    sconT = np.ascontiguousarray(s_con_b.T)          # [j, i]
    sconT1 = sconT[0:128].copy()
    sconT2p = np.concatenate([sconT[128:192, 0:96], sconT[128:192, 96:192]], 0).copy()
    return {"c1": c1, "c2": c2, "siga": siga, "sigb": sigb,
            "sconT1": sconT1, "sconT2p": sconT2p}


def kernel(s_con, s_bin, mask):
    from concourse.bass_utils import run_bass_kernel_spmd

    s_con = np.asarray(s_con, dtype=np.float32)
    s_bin = np.asarray(s_bin, dtype=np.float32)
    mask = np.asarray(mask)

    idx = np.arange(S)
    ne = idx[:, None] != idx[None, :]                       # [a, k]
    m2 = ne[:, None, :] & ne[None, :, :]                    # [i, j, k]
    full_mask = mask[:, :, :, None] & m2[None]              # [B, i, j, k]
    sbm16 = (s_bin * full_mask).astype(np.float16)

    nc = _get_program()
    in_maps = [_prep_core_inputs(s_con[b], sbm16[b]) for b in range(B)]
    res = run_bass_kernel_spmd(nc, in_maps, list(range(B)))
    out = np.stack([res.results[b]["qt_out"].T for b in range(B)], 0)
    return np.ascontiguousarray(out.astype(np.float32))



# revision 2
# speedup vs baseline: 4.5996x; 4.5996x over previous
"""Trainium2 Bass kernel for MFVIConstituency mean-field iterations.

Per batch b (one NeuronCore each, 8 total):
    q = s_con;  repeat 3x:  q[i,j] = s_con[i,j] + sum_k sig(q)[j,k] * sb[i,j,k]
    out = sigmoid(q)
where sb = s_bin * mask2o, mask2o[i,j,k] = mask[i,j] & (i!=k) & (j!=k).

Strategy: the contraction sum_k sig(q)[j,k]*sb[i,j,k] is, for each fixed j, a
matvec with a j-dependent matrix -- so it runs on the otherwise-idle PE array
as 192 per-j matvec groups per iteration: stationary = sb[:,j,:]^T (fp16,
k on partitions, i on the stationary free dim), moving = one column of
sig(q)^T, accumulating k-chunks (128 + 64+1) into PSUM in fp32.  The s_con
add is folded in as a 193rd contraction row whose moving value is 1.0 and
whose stationary row holds s_con[:,j].  Boundaries between iterations:
ACT sigmoid (PSUM f32 -> SBUF fp16), PE transpose via identity (fp16 PSUM),
DVE copy back to SBUF.  The 14.2 MB weight cache streams in as j-chunked
DMAs which iteration 1 consumes as they arrive.

Host (numpy) does input prep only: masking, fp16 cast, [k,j,i] layout
packing, and the iteration-1 sigmoid of s_con (an input-operand transform,
same as the previous kernel).
"""

import numpy as np

S = 192
B = 8
P = 128
K1 = 65            # second k-chunk: 64 sb rows + 1 s_con row
NJC = 12           # DMA chunks over j for load/compute overlap
JC = S // NJC      # 16 j per chunk

_CACHE = {}


def _build_program():
    import concourse.tile as tile
    from concourse import mybir, bacc
    from contextlib import ExitStack

    f32, f16 = mybir.dt.float32, mybir.dt.float16
    Sig = mybir.ActivationFunctionType.Sigmoid

    nc = bacc.Bacc("TRN2", target_bir_lowering=False, debug=False, num_devices=B)

    c0_d = nc.dram_tensor("c0", [P, S * S], f16, kind="ExternalInput")
    c1x_d = nc.dram_tensor("c1x", [K1, S * S], f16, kind="ExternalInput")
    st0_d = nc.dram_tensor("st0", [P, S], f16, kind="ExternalInput")
    st1x_d = nc.dram_tensor("st1x", [K1, S], f16, kind="ExternalInput")
    id_d = nc.dram_tensor("idm", [P, P], f16, kind="ExternalInput")
    qout_d = nc.dram_tensor("qout", [S, S], f32, kind="ExternalOutput")

    with tile.TileContext(nc) as tc, ExitStack() as ctx:
        cache_p = ctx.enter_context(tc.tile_pool(name="cache", bufs=1))
        sig_p = ctx.enter_context(tc.tile_pool(name="sig", bufs=2))
        out_p = ctx.enter_context(tc.tile_pool(name="out", bufs=1))
        qp_p = ctx.enter_context(tc.tile_pool(name="qp", bufs=2, space="PSUM"))
        tp_p = ctx.enter_context(tc.tile_pool(name="tp", bufs=1, space="PSUM"))

        SBT0 = cache_p.tile([P, S * S], f16, tag="sbt0")
        SBT1x = cache_p.tile([K1, S * S], f16, tag="sbt1")
        SIGT0 = cache_p.tile([P, S], f16, tag="sigt0")
        SIGT1x = cache_p.tile([K1, S], f16, tag="sigt1")
        ID = cache_p.tile([P, P], f16, tag="id")

        nc.sync.dma_start(SIGT0[:], st0_d.ap())
        nc.sync.dma_start(SIGT1x[:], st1x_d.ap())
        nc.sync.dma_start(ID[:], id_d.ap())
        for c in range(NJC):
            lo, hi = c * JC * S, (c + 1) * JC * S
            nc.sync.dma_start(SBT0[:, lo:hi], c0_d.ap()[:, lo:hi])
            nc.scalar.dma_start(SBT1x[:, lo:hi], c1x_d.ap()[:, lo:hi])

        for it in range(3):
            qp0 = qp_p.tile([P, S], f32, tag="qp0")
            qp1 = qp_p.tile([64, S], f32, tag="qp1")
            for j in range(S):
                b0 = j * S
                sgt0 = SIGT0[:, j:j + 1]
                sgt1 = SIGT1x[:, j:j + 1]
                nc.tensor.matmul(qp0[:, j:j + 1], SBT0[:, b0:b0 + 128],
                                 sgt0, start=True, stop=False)
                nc.tensor.matmul(qp0[:, j:j + 1], SBT1x[:, b0:b0 + 128],
                                 sgt1, start=False, stop=True)
                nc.tensor.matmul(qp1[:, j:j + 1], SBT0[:, b0 + 128:b0 + 192],
                                 sgt0, start=True, stop=False)
                nc.tensor.matmul(qp1[:, j:j + 1], SBT1x[:, b0 + 128:b0 + 192],
                                 sgt1, start=False, stop=True)
            if it < 2:
                SIG0 = sig_p.tile([P, S], f16, tag="sig0")
                SIG1 = sig_p.tile([64, S], f16, tag="sig1")
                nc.scalar.activation(SIG0[:], qp0[:], Sig)
                nc.scalar.activation(SIG1[:], qp1[:], Sig)
                # full transpose of sigmoid(q): sig(q)[a,b] -> SIGT[b,a]
                tp1 = tp_p.tile([P, P], f16, tag="tp1")
                tp3 = tp_p.tile([64, P], f16, tag="tp3")
                tp2 = tp_p.tile([P, 64], f16, tag="tp2")
                tp4 = tp_p.tile([64, 64], f16, tag="tp4")
                nc.tensor.transpose(tp1[:], SIG0[:, 0:128], ID[:])
                nc.tensor.transpose(tp3[:], SIG0[:, 128:192], ID[:])
                nc.tensor.transpose(tp2[:], SIG1[:, 0:128], ID[0:64, 0:64])
                nc.tensor.transpose(tp4[:], SIG1[:, 128:192], ID[0:64, 0:64])
                nc.vector.tensor_copy(SIGT0[:, 0:128], tp1[:])
                nc.vector.tensor_copy(SIGT1x[0:64, 0:128], tp3[:])
                nc.vector.tensor_copy(SIGT0[:, 128:192], tp2[:])
                nc.vector.tensor_copy(SIGT1x[0:64, 128:192], tp4[:])
            else:
                o0 = out_p.tile([P, S], f32, tag="o0")
                o1 = out_p.tile([64, S], f32, tag="o1")
                nc.scalar.activation(o0[:], qp0[:], Sig)
                nc.sync.dma_start(qout_d.ap()[0:128, :], o0[:])
                nc.scalar.activation(o1[:], qp1[:], Sig)
                nc.sync.dma_start(qout_d.ap()[128:192, :], o1[:])
    nc.compile()
    return nc


def _get_program():
    if "nc" not in _CACHE:
        _CACHE["nc"] = _build_program()
    return _CACHE["nc"]


def _prep_core_inputs(s_con_b, sbm16_b):
    """Per-batch input dict. sbm16_b: masked s_bin, fp16, [i, j, k]."""
    T = sbm16_b.transpose(2, 1, 0)                  # [k, j, i]
    c0 = np.ascontiguousarray(T[0:128]).reshape(P, S * S)
    sconT = np.ascontiguousarray(s_con_b.T).astype(np.float16)  # [j, i]
    c1x = np.concatenate(
        [np.ascontiguousarray(T[128:192]).reshape(64, S * S),
         sconT.reshape(1, S * S)], 0)
    sig1T = (1.0 / (1.0 + np.exp(-s_con_b))).astype(np.float16).T  # [k, j]
    st0 = np.ascontiguousarray(sig1T[0:128])
    st1x = np.concatenate(
        [np.ascontiguousarray(sig1T[128:192]),
         np.ones((1, S), dtype=np.float16)], 0)
    idm = np.eye(P, dtype=np.float16)
    return {"c0": c0, "c1x": c1x, "st0": st0, "st1x": st1x, "idm": idm}


def kernel(s_con, s_bin, mask):
    from concourse.bass_utils import run_bass_kernel_spmd

    s_con = np.asarray(s_con, dtype=np.float32)
    s_bin = np.asarray(s_bin, dtype=np.float32)
    mask = np.asarray(mask)

    idx = np.arange(S)
    ne = idx[:, None] != idx[None, :]                       # [a, k]
    m2 = ne[:, None, :] & ne[None, :, :]                    # [i, j, k]
    full_mask = mask[:, :, :, None] & m2[None]              # [B, i, j, k]
    sbm16 = (s_bin * full_mask).astype(np.float16)

    nc = _get_program()
    in_maps = [_prep_core_inputs(s_con[b], sbm16[b]) for b in range(B)]
    res = run_bass_kernel_spmd(nc, in_maps, list(range(B)))
    out = np.stack([res.results[b]["qout"] for b in range(B)], 0)
    return np.ascontiguousarray(out.astype(np.float32))


# revision 19
# speedup vs baseline: 6.4904x; 1.4111x over previous
"""Trainium2 Bass kernel for MFVIConstituency mean-field iterations.

Per batch b (one NeuronCore each, 8 total):
    q = s_con;  repeat 3x:  q[i,j] = s_con[i,j] + sum_k sig(q)[j,k] * sb[i,j,k]
    out = sigmoid(q)
where sb = s_bin * mask2o, mask2o[i,j,k] = mask[i,j] & (i!=k) & (j!=k).

Strategy: the contraction sum_k sig(q)[j,k]*sb[i,j,k] is, for each fixed j, a
matvec with a j-dependent matrix -- so it runs on the otherwise-idle PE array
as per-j matvec groups: stationary = sb[:,j,:]^T (fp16, k on partitions, i on
the stationary free dim), moving = one column of sig(q)^T, accumulating the
two k-chunks (128 and 64+1) into PSUM in fp32.  The s_con add is folded in as
a 193rd contraction row whose moving value is 1.0 and whose stationary row
holds s_con[:,j].

The 14.2MB weight cache streams in over all three DMA-capable queues
(SP/ACT/Pool) as interleaved j-chunks; iteration 1 consumes them as they
land (its groups are per-j: start+stop adjacent).  Iterations 2-3 are
emitted as two passes with interleaved PSUM groups (skip_group_check):
pass A all k0 matmuls (start), pass B all k1 matmuls (stop).  Since the
next iteration's k0 moving operand (SIGT0 = rows 0:128 of sig(q)^T) depends
only on q columns 0:128 and the k1 operand only on columns 128:192, the
sigmoid halves (ACT), PE transposes (via identity), and copy-backs (DVE +
Pool) chase pass B's tail and the PE barely stalls between iterations.
The sigmoid act table is preloaded by a warmup activation under the DMA.

Host (numpy) does input prep only: masking, fp16 cast, [k,j,i] layout
packing, and the iteration-1 sigmoid of s_con (an input-operand transform).
"""

import numpy as np

S = 192
B = 8
P = 128
SS = S * S            # 36864
H0 = P + S            # c0 header: identity(128) | st0(192)
H1 = S                # c1x header: st1x(192)
C0W = H0 + SS
C1W = H1 + SS
import os
NJC = int(os.environ.get("NJC", "24"))   # DMA j-chunks (8 j per chunk)
JC = S // NJC         # 16

_CACHE = {}


def _build_program():
    import concourse.tile as tile
    from concourse import mybir, bacc
    from contextlib import ExitStack

    f32, f16 = mybir.dt.float32, mybir.dt.float16
    Sig = mybir.ActivationFunctionType.Sigmoid

    nc = bacc.Bacc("TRN2", target_bir_lowering=False, debug=False, num_devices=B)

    c0_d = nc.dram_tensor("c0", [P, C0W], f16, kind="ExternalInput")
    c1x_d = nc.dram_tensor("c1x", [65, C1W], f16, kind="ExternalInput")
    qout_d = nc.dram_tensor("qout", [S, S], f32, kind="ExternalOutput")

    with tile.TileContext(nc) as tc, ExitStack() as ctx:
        cache_p = ctx.enter_context(tc.tile_pool(name="cache", bufs=1))
        sig_p = ctx.enter_context(tc.tile_pool(name="sig", bufs=2))
        out_p = ctx.enter_context(tc.tile_pool(name="out", bufs=1))
        warm_p = ctx.enter_context(tc.tile_pool(name="warm", bufs=1))
        qp_p = ctx.enter_context(tc.tile_pool(name="qp", bufs=2, space="PSUM"))
        tp_p = ctx.enter_context(tc.tile_pool(name="tp", bufs=1, space="PSUM"))

        SBT0 = cache_p.tile([P, C0W], f16, tag="sbt0")
        SBT1x = cache_p.tile([65, C1W], f16, tag="sbt1")
        ID = SBT0[:, 0:P]                   # eye(128)
        SIGT0 = SBT0[:, P:H0]               # sig(q)^T rows 0:128, col j
        SIGT1x = SBT1x[:, 0:H1]             # rows 128:192 + ones row

        def w0(j):                          # stationary slice base in SBT0/SBT1x
            return H0 + j * S

        def w1(j):
            return H1 + j * S

        warm = warm_p.tile([1, 2], f16, tag="warm")
        wsig = warm_p.tile([1, 2], f16, tag="wsig")

        # chunked round-robin DMA across the 3 queues (2 concurrent in flight);
        # chunk 0 carries the identity + iteration-1 sigmoid operands up front
        qs = [nc.sync, nc.scalar, nc.gpsimd]

        def dma_round(c):
            lo0 = 0 if c == 0 else H0 + c * JC * S
            lo1 = 0 if c == 0 else H1 + c * JC * S
            hi0, hi1 = H0 + (c + 1) * JC * S, H1 + (c + 1) * JC * S
            qs[(2 * c) % 3].dma_start(SBT0[:, lo0:hi0], c0_d.ap()[:, lo0:hi0])
            qs[(2 * c + 1) % 3].dma_start(SBT1x[:, lo1:hi1], c1x_d.ap()[:, lo1:hi1])

        def mm4(qp0, qp1, j):
            b0, b1 = w0(j), w1(j)
            sg0, sg1 = SIGT0[:, j:j + 1], SIGT1x[:, j:j + 1]
            nc.tensor.matmul(qp0[:, j:j + 1], SBT0[:, b0:b0 + 128], sg0,
                             start=True, stop=False, skip_group_check=True)
            nc.tensor.matmul(qp0[:, j:j + 1], SBT1x[:, b1:b1 + 128], sg1,
                             start=False, stop=True, skip_group_check=True)
            nc.tensor.matmul(qp1[:, j:j + 1], SBT0[:, b0 + 128:b0 + 192], sg0,
                             start=True, stop=False, skip_group_check=True)
            nc.tensor.matmul(qp1[:, j:j + 1], SBT1x[:, b1 + 128:b1 + 192], sg1,
                             start=False, stop=True, skip_group_check=True)

        def bnd_tiles():
            SIG0 = sig_p.tile([P, S], f16, tag="sig0")
            SIG1 = sig_p.tile([64, S], f16, tag="sig1")
            tp1 = tp_p.tile([P, P], f16, tag="tp1")
            tp2 = tp_p.tile([P, 64], f16, tag="tp2")
            tp3 = tp_p.tile([64, P], f16, tag="tp3")
            tp4 = tp_p.tile([64, 64], f16, tag="tp4")
            return SIG0, SIG1, tp1, tp2, tp3, tp4

        def bnd_lo(qp0, qp1, T, copies=True):
            # sigmoid + transpose for q columns 0:128 (feeds SIGT cols 0:128)
            SIG0, SIG1, tp1, tp2, tp3, tp4 = T
            nc.scalar.activation(SIG0[:, 0:128], qp0[:, 0:128], Sig)
            nc.scalar.activation(SIG1[:, 0:128], qp1[:, 0:128], Sig)
            nc.tensor.transpose(tp1[:], SIG0[:, 0:128], ID)
            nc.tensor.transpose(tp2[:], SIG1[:, 0:128], ID[0:64, 0:64])
            if copies:
                bnd_lo_copies(T)

        def bnd_lo_copies(T):
            SIG0, SIG1, tp1, tp2, tp3, tp4 = T
            nc.vector.tensor_copy(SIGT0[:, 0:128], tp1[:])
            nc.vector.tensor_copy(SIGT0[:, 128:192], tp2[:])

        def bnd_hi(qp0, qp1, T):
            # sigmoid + transpose for q columns 128:192 (feeds SIGT1x)
            SIG0, SIG1, tp1, tp2, tp3, tp4 = T
            nc.scalar.activation(SIG0[:, 128:192], qp0[:, 128:192], Sig)
            nc.scalar.activation(SIG1[:, 128:192], qp1[:, 128:192], Sig)
            nc.tensor.transpose(tp3[:], SIG0[:, 128:192], ID)
            nc.tensor.transpose(tp4[:], SIG1[:, 128:192], ID[0:64, 0:64])
            nc.vector.tensor_copy(SIGT1x[0:64, 0:128], tp3[:])
            nc.vector.tensor_copy(SIGT1x[0:64, 128:192], tp4[:])

        # --- iteration 1: per-j groups ride the DMA stream ------------------
        qpA0 = qp_p.tile([P, S], f32, tag="qp0")
        qpA1 = qp_p.tile([64, S], f32, tag="qp1")
        for c in range(NJC):
            dma_round(c)
        nc.vector.memset(warm[:], 0.0)
        nc.scalar.activation(wsig[:], warm[:], Sig)   # preload sigmoid table
        for j in range(S):
            mm4(qpA0, qpA1, j)
        TA = bnd_tiles()
        bnd_lo(qpA0, qpA1, TA)
        bnd_hi(qpA0, qpA1, TA)

        # --- iterations 2..3: pass-split pipeline ---------------------------
        for it in (1, 2):
            qp0 = qp_p.tile([P, S], f32, tag="qp0")
            qp1 = qp_p.tile([64, S], f32, tag="qp1")
            for j in range(S):
                mm4(qp0, qp1, j)
            if it < 2:
                T = bnd_tiles()
                bnd_lo(qp0, qp1, T)
                bnd_hi(qp0, qp1, T)
            else:
                o0 = out_p.tile([P, S], f32, tag="o0")
                o1 = out_p.tile([64, S], f32, tag="o1")
                for qlo in range(0, S, 96):   # half-sliced output tail
                    qhi = qlo + 96
                    nc.scalar.activation(o0[:, qlo:qhi], qp0[:, qlo:qhi], Sig)
                    nc.scalar.activation(o1[:, qlo:qhi], qp1[:, qlo:qhi], Sig)
                    nc.sync.dma_start(qout_d.ap()[0:128, qlo:qhi], o0[:, qlo:qhi])
                    nc.gpsimd.dma_start(qout_d.ap()[128:192, qlo:qhi], o1[:, qlo:qhi])
    nc.compile()
    return nc


def _get_program():
    if "nc" not in _CACHE:
        _CACHE["nc"] = _build_program()
    return _CACHE["nc"]


def _prep_core_inputs(s_con_b, sbm16_b):
    """Per-batch input dict. sbm16_b: masked s_bin, fp16, [i, j, k]."""
    T = sbm16_b.transpose(2, 1, 0)                  # [k, j, i]
    sconT = np.ascontiguousarray(s_con_b.T).astype(np.float16)   # [j, i]
    sig1T = (1.0 / (1.0 + np.exp(-s_con_b))).astype(np.float16).T  # [k, j]
    c0 = np.concatenate(
        [np.eye(P, dtype=np.float16),
         np.ascontiguousarray(sig1T[0:128]),
         np.ascontiguousarray(T[0:128]).reshape(P, SS)], 1)
    c1x = np.concatenate(
        [np.concatenate([np.ascontiguousarray(sig1T[128:192]),
                         np.ones((1, S), dtype=np.float16)], 0),
         np.concatenate([np.ascontiguousarray(T[128:192]).reshape(64, SS),
                         sconT.reshape(1, SS)], 0)], 1)
    return {"c0": c0, "c1x": c1x}


def kernel(s_con, s_bin, mask):
    from concourse.bass_utils import run_bass_kernel_spmd

    s_con = np.asarray(s_con, dtype=np.float32)
    s_bin = np.asarray(s_bin, dtype=np.float32)
    mask = np.asarray(mask)

    idx = np.arange(S)
    ne = idx[:, None] != idx[None, :]                       # [a, k]
    m2 = ne[:, None, :] & ne[None, :, :]                    # [i, j, k]
    full_mask = mask[:, :, :, None] & m2[None]              # [B, i, j, k]
    sbm16 = (s_bin * full_mask).astype(np.float16)

    nc = _get_program()
    in_maps = [_prep_core_inputs(s_con[b], sbm16[b]) for b in range(B)]
    res = run_bass_kernel_spmd(nc, in_maps, list(range(B)))
    out = np.stack([res.results[b]["qout"] for b in range(B)], 0)
    return np.ascontiguousarray(out.astype(np.float32))


# revision 24
# speedup vs baseline: 6.6537x; 1.0251x over previous
"""Trainium2 Bass kernel for MFVIConstituency mean-field iterations.

Per batch b (one NeuronCore each, 8 total):
    q = s_con;  repeat 3x:  q[i,j] = s_con[i,j] + sum_k sig(q)[j,k] * sb[i,j,k]
    out = sigmoid(q)
where sb = s_bin * mask2o, mask2o[i,j,k] = mask[i,j] & (i!=k) & (j!=k).

Strategy: the contraction sum_k sig(q)[j,k]*sb[i,j,k] is, for each fixed j, a
matvec with a j-dependent matrix -- so it runs on the otherwise-idle PE array
as per-j matvec groups: stationary = sb[:,j,:]^T (fp16, k on partitions, i on
the stationary free dim), moving = one column of sig(q)^T, accumulating the
two k-chunks (128 and 64+1) into PSUM in fp32.  The s_con add is folded in as
a 193rd contraction row whose moving value is 1.0 and whose stationary row
holds s_con[:,j].

The 14.2MB weight cache streams in over all three DMA-capable queues
(SP/ACT/Pool) as interleaved j-chunks; iteration 1 consumes them as they
land (its groups are per-j: start+stop adjacent).  Iterations 2-3 are
emitted as two passes with interleaved PSUM groups (skip_group_check):
pass A all k0 matmuls (start), pass B all k1 matmuls (stop).  Since the
next iteration's k0 moving operand (SIGT0 = rows 0:128 of sig(q)^T) depends
only on q columns 0:128 and the k1 operand only on columns 128:192, the
sigmoid halves (ACT), PE transposes (via identity), and copy-backs (DVE +
Pool) chase pass B's tail and the PE barely stalls between iterations.
The sigmoid act table is preloaded by a warmup activation under the DMA.

Host (numpy) does input prep only: masking, fp16 cast, [k,j,i] layout
packing, and the iteration-1 sigmoid of s_con (an input-operand transform).
"""

import numpy as np

S = 192
B = 8
P = 128
SS = S * S            # 36864
H0 = P + S            # c0 header: identity(128) | st0(192)
H1 = S                # c1x header: st1x(192)
C0W = H0 + SS
C1W = H1 + SS
import os
NJC = int(os.environ.get("NJC", "24"))   # DMA j-chunks (8 j per chunk)
JC = S // NJC         # 16

_CACHE = {}


def _build_program():
    import concourse.tile as tile
    from concourse import mybir, bacc
    from contextlib import ExitStack

    f32, f16 = mybir.dt.float32, mybir.dt.float16
    Sig = mybir.ActivationFunctionType.Sigmoid

    nc = bacc.Bacc("TRN2", target_bir_lowering=False, debug=False, num_devices=B)

    c0_d = nc.dram_tensor("c0", [P, C0W], f16, kind="ExternalInput")
    c1x_d = nc.dram_tensor("c1x", [65, C1W], f16, kind="ExternalInput")
    qout_d = nc.dram_tensor("qout", [S, S], f32, kind="ExternalOutput")

    with tile.TileContext(nc) as tc, ExitStack() as ctx:
        cache_p = ctx.enter_context(tc.tile_pool(name="cache", bufs=1))
        sig_p = ctx.enter_context(tc.tile_pool(name="sig", bufs=2))
        out_p = ctx.enter_context(tc.tile_pool(name="out", bufs=1))
        warm_p = ctx.enter_context(tc.tile_pool(name="warm", bufs=1))
        qp_p = ctx.enter_context(tc.tile_pool(name="qp", bufs=2, space="PSUM"))
        tp_p = ctx.enter_context(tc.tile_pool(name="tp", bufs=1, space="PSUM"))

        SBT0 = cache_p.tile([P, C0W], f16, tag="sbt0")
        SBT1x = cache_p.tile([65, C1W], f16, tag="sbt1")
        ID = SBT0[:, 0:P]                   # eye(128)
        SIGT0 = SBT0[:, P:H0]               # sig(q)^T rows 0:128, col j
        SIGT1x = SBT1x[:, 0:H1]             # rows 128:192 + ones row

        def w0(j):                          # stationary slice base in SBT0/SBT1x
            return H0 + j * S

        def w1(j):
            return H1 + j * S

        warm = warm_p.tile([1, 2], f16, tag="warm")
        wsig = warm_p.tile([1, 2], f16, tag="wsig")

        # chunked round-robin DMA across the 3 queues (2 concurrent in flight);
        # chunk 0 carries the identity + iteration-1 sigmoid operands up front
        qs = [nc.sync, nc.scalar, nc.gpsimd]

        def dma_round(c):
            lo0 = 0 if c == 0 else H0 + c * JC * S
            lo1 = 0 if c == 0 else H1 + c * JC * S
            hi0, hi1 = H0 + (c + 1) * JC * S, H1 + (c + 1) * JC * S
            qs[(2 * c) % 3].dma_start(SBT0[:, lo0:hi0], c0_d.ap()[:, lo0:hi0])
            qs[(2 * c + 1) % 3].dma_start(SBT1x[:, lo1:hi1], c1x_d.ap()[:, lo1:hi1])

        def mm4(qp0, qp1, j):
            b0, b1 = w0(j), w1(j)
            sg0, sg1 = SIGT0[:, j:j + 1], SIGT1x[:, j:j + 1]
            nc.tensor.matmul(qp0[:, j:j + 1], SBT0[:, b0:b0 + 128], sg0,
                             start=True, stop=False, skip_group_check=True)
            nc.tensor.matmul(qp0[:, j:j + 1], SBT1x[:, b1:b1 + 128], sg1,
                             start=False, stop=True, skip_group_check=True)
            nc.tensor.matmul(qp1[:, j:j + 1], SBT0[:, b0 + 128:b0 + 192], sg0,
                             start=True, stop=False, skip_group_check=True)
            nc.tensor.matmul(qp1[:, j:j + 1], SBT1x[:, b1 + 128:b1 + 192], sg1,
                             start=False, stop=True, skip_group_check=True)

        def bnd_tiles():
            SIG0 = sig_p.tile([P, S], f16, tag="sig0")
            SIG1 = sig_p.tile([64, S], f16, tag="sig1")
            tp1 = tp_p.tile([P, P], f16, tag="tp1")
            tp2 = tp_p.tile([P, 64], f16, tag="tp2")
            tp3 = tp_p.tile([64, P], f16, tag="tp3")
            tp4 = tp_p.tile([64, 64], f16, tag="tp4")
            return SIG0, SIG1, tp1, tp2, tp3, tp4

        def bnd_lo(qp0, qp1, T):
            # sigmoid + transpose for q columns 0:128 (feeds SIGT cols 0:128);
            # copies deferred to bnd_hi so old-operand readers stay correct
            SIG0, SIG1, tp1, tp2, tp3, tp4 = T
            nc.scalar.activation(SIG0[:, 0:128], qp0[:, 0:128], Sig)
            nc.scalar.activation(SIG1[:, 0:128], qp1[:, 0:128], Sig)
            nc.tensor.transpose(tp1[:], SIG0[:, 0:128], ID)
            nc.tensor.transpose(tp2[:], SIG1[:, 0:128], ID[0:64, 0:64])

        def bnd_hi(qp0, qp1, T):
            # sigmoid + transpose for q columns 128:192 (feeds SIGT1x)
            SIG0, SIG1, tp1, tp2, tp3, tp4 = T
            nc.scalar.activation(SIG0[:, 128:192], qp0[:, 128:192], Sig)
            nc.scalar.activation(SIG1[:, 128:192], qp1[:, 128:192], Sig)
            nc.tensor.transpose(tp3[:], SIG0[:, 128:192], ID)
            nc.tensor.transpose(tp4[:], SIG1[:, 128:192], ID[0:64, 0:64])
            nc.vector.tensor_copy(SIGT0[:, 0:128], tp1[:])
            nc.vector.tensor_copy(SIGT1x[0:64, 0:128], tp3[:])
            nc.vector.tensor_copy(SIGT0[:, 128:192], tp2[:])
            nc.vector.tensor_copy(SIGT1x[0:64, 128:192], tp4[:])

        # --- iteration 1: per-j groups ride the DMA stream ------------------
        qpA0 = qp_p.tile([P, S], f32, tag="qp0")
        qpA1 = qp_p.tile([64, S], f32, tag="qp1")
        for c in range(NJC):
            dma_round(c)
        nc.vector.memset(warm[:], 0.0)
        nc.scalar.activation(wsig[:], warm[:], Sig)   # preload sigmoid table
        for j in range(S):
            mm4(qpA0, qpA1, j)
        # boundary 1: start is hard-gated (ACT busy with DMA until the ladder
        # ends), so two full-size sigmoids beat four halves; copies ordered so
        # iteration 2's first columns unblock first
        TA = bnd_tiles()
        SIG0, SIG1, tp1, tp2, tp3, tp4 = TA
        nc.scalar.activation(SIG0[:], qpA0[:], Sig)
        nc.scalar.activation(SIG1[:], qpA1[:], Sig)
        nc.tensor.transpose(tp1[:], SIG0[:, 0:128], ID)
        nc.tensor.transpose(tp3[:], SIG0[:, 128:192], ID)
        nc.tensor.transpose(tp2[:], SIG1[:, 0:128], ID[0:64, 0:64])
        nc.tensor.transpose(tp4[:], SIG1[:, 128:192], ID[0:64, 0:64])
        nc.vector.tensor_copy(SIGT0[:, 0:128], tp1[:])
        nc.vector.tensor_copy(SIGT1x[0:64, 0:128], tp3[:])
        nc.vector.tensor_copy(SIGT0[:, 128:192], tp2[:])
        nc.vector.tensor_copy(SIGT1x[0:64, 128:192], tp4[:])

        # --- iterations 2..3: pass-split pipeline ---------------------------
        for it in (1, 2):
            qp0 = qp_p.tile([P, S], f32, tag="qp0")
            qp1 = qp_p.tile([64, S], f32, tag="qp1")
            for j in range(S):
                mm4(qp0, qp1, j)
            if it < 2:
                T = bnd_tiles()
                bnd_lo(qp0, qp1, T)
                bnd_hi(qp0, qp1, T)
            else:
                o0 = out_p.tile([P, S], f32, tag="o0")
                o1 = out_p.tile([64, S], f32, tag="o1")
                nc.scalar.activation(o1[:], qp1[:], Sig)
                nc.gpsimd.dma_start(qout_d.ap()[128:192, :], o1[:])
                nc.scalar.activation(o0[:], qp0[:], Sig)
                nc.sync.dma_start(qout_d.ap()[0:128, :], o0[:])
    nc.compile()
    return nc


def _get_program():
    if "nc" not in _CACHE:
        _CACHE["nc"] = _build_program()
    return _CACHE["nc"]


def _prep_core_inputs(s_con_b, sbm16_b):
    """Per-batch input dict. sbm16_b: masked s_bin, fp16, [i, j, k]."""
    T = sbm16_b.transpose(2, 1, 0)                  # [k, j, i]
    sconT = np.ascontiguousarray(s_con_b.T).astype(np.float16)   # [j, i]
    sig1T = (1.0 / (1.0 + np.exp(-s_con_b))).astype(np.float16).T  # [k, j]
    c0 = np.concatenate(
        [np.eye(P, dtype=np.float16),
         np.ascontiguousarray(sig1T[0:128]),
         np.ascontiguousarray(T[0:128]).reshape(P, SS)], 1)
    c1x = np.concatenate(
        [np.concatenate([np.ascontiguousarray(sig1T[128:192]),
                         np.ones((1, S), dtype=np.float16)], 0),
         np.concatenate([np.ascontiguousarray(T[128:192]).reshape(64, SS),
                         sconT.reshape(1, SS)], 0)], 1)
    return {"c0": c0, "c1x": c1x}


def kernel(s_con, s_bin, mask):
    from concourse.bass_utils import run_bass_kernel_spmd

    s_con = np.asarray(s_con, dtype=np.float32)
    s_bin = np.asarray(s_bin, dtype=np.float32)
    mask = np.asarray(mask)

    idx = np.arange(S)
    ne = idx[:, None] != idx[None, :]                       # [a, k]
    m2 = ne[:, None, :] & ne[None, :, :]                    # [i, j, k]
    full_mask = mask[:, :, :, None] & m2[None]              # [B, i, j, k]
    sbm16 = (s_bin * full_mask).astype(np.float16)

    nc = _get_program()
    in_maps = [_prep_core_inputs(s_con[b], sbm16[b]) for b in range(B)]
    res = run_bass_kernel_spmd(nc, in_maps, list(range(B)))
    out = np.stack([res.results[b]["qout"] for b in range(B)], 0)
    return np.ascontiguousarray(out.astype(np.float32))
